# revision 36
# baseline (speedup 1.0000x reference)
"""Trainium2 Bass kernel for nn_AttentionBlock (GroupNorm + spatial self-attention
+ residual). Full inputs in, full outputs out; data-parallel over batch (B=8)
across 8 NeuronCores; each core processes one [C=256, N=4096] image.

Design (full-fp8 attention; see study.py for the precision budget):
  - All big matmuls run fp8-e4m3 DoubleRow (0.5 cycles/row, 4x less PE time
    than two bf16 chunk matmuls for a 256-contraction). Weights are scaled
    x16 on the host so their sigma~1 lands in fp8's normal range; the score
    scale absorbs 1/256 and the softmax denominator absorbs the v-side 16.
  - DR layouts are [128, 2, n] with channel = r*128 + p.
  - q = 16(q_mm + bq) fp8; k = 16 k_mm fp8 (k-bias provably cancels in the
    softmax over j); v = 16 v_mm fp8 transposed to [n, c] pairs (v-bias passes
    through softmax and is folded into proj_b on the host).
  - scores s' = k'^T q' = 256 s, PSUM f32; w = exp(s'*SCALE/256 - 4) in fp8:
    the e^-4 offset keeps fp8 in range and cancels in normalization. exp runs
    split across two engines: ACT (exact exp, fp8 out) for the first
    N_ACT_PAIRS j-pairs of each stripe, DVE for the rest via a Schraudolph
    bit-trick (uint8 = rne(8/ln2*u + 56) is the fp8-e4m3 encoding of e^u;
    DVE's f32->uint8 convert rounds+saturates, verified on HW).
  - softmax denominator: DR ones-matmul (value 16 = v descale) accumulating
    [1, SW] in PSUM; reciprocal_approx_fast on DVE; partition_broadcast on
    the (otherwise idle) Pool engine; a = a_ps * rinv -> fp8 pairs (DVE).
  - proj fp8 DR with the residual accumulated into the same PSUM group via a
    16*I bf16 identity-matmul of xbf; o = Identity(o_ps/16 + projb') on ACT
    writes the final output, DMA'd on the SP HWDGE queue.
  - PSUM: 5 per-jt score banks + 2 AV banks + 1 bank shared by the
    denominator and o_ps (temporally disjoint, same pool tag). PE writes into
    a shared bank via a second aliased matmul fail BIR verification, so the
    rinv broadcast stays on Pool rather than a PE rank-1 matmul.
"""

import sys

try:
    import concourse  # noqa: F401
except ImportError:
    sys.path.insert(0, "/opt/trn_rl_repo")

import numpy as np
import ml_dtypes

import bass_rust as _bass_rust
import concourse.bacc as bacc
import concourse.tile as tile
from concourse import mybir
from concourse import bass_isa
from concourse.bass_utils import run_bass_kernel_spmd

F32 = mybir.dt.float32
BF16 = mybir.dt.bfloat16
FP8 = mybir.dt.float8e4
U8 = mybir.dt.uint8
AF = mybir.ActivationFunctionType
ALU = mybir.AluOpType
AX = mybir.AxisListType
DR = mybir.MatmulPerfMode.DoubleRow

C = 256          # channels
N = 4096         # spatial positions
GROUPS = 32
EPS = 1e-5
SCALE = C ** -0.5
SC2 = SCALE / 256.0          # fold the x16 q/k weight scaling out of scores
OFFSET = -4.0                # exp offset; cancels in softmax normalization
SCHRA_A = 8.0 / np.log(2.0)  # fp8-e4m3 schraudolph slope
NSTRIPE = 8
SW = N // NSTRIPE            # 512
NPAIR = N // 256             # 16 j-pairs (j-tiles of 128, two per pair)
PLAG = 6                     # pairs of lag between exp production and AV/ones
NJT = N // 128               # 32 j-tiles per stripe
# exp engine per j-tile, spread so ACT (faster, ~18 tiles) and DVE (~14) run
# concurrently within a stripe (a contiguous split serializes them through
# the score-buffer WAR).
ACT_JTS = frozenset(j for j in range(NJT) if (j * 18) // NJT != ((j + 1) * 18) // NJT)
GSIZE = (C // GROUPS) * N


def _emit(nc, tc, d):
    const = tc.alloc_tile_pool(name="const", bufs=1)

    # --- input DMAs: bf16 x split across both HWDGE queues, then weights ---
    xbf = []
    for t in range(2):
        xb_ = const.tile([128, N], BF16, tag=f"xbf{t}", name=f"xbf{t}")
        for g, eng in enumerate((nc.sync, nc.scalar)):
            eng.dma_start(xb_[:, g * (N // 2):(g + 1) * (N // 2)],
                          d["xbf"][t * 128:(t + 1) * 128,
                                   g * (N // 2):(g + 1) * (N // 2)])
        xbf.append(xb_)

    w8 = const.tile([128, 2, 1024], FP8, tag="w8")
    nc.scalar.dma_start(w8[:].bitcast(U8), d["w8"][:])
    wq8 = w8[:, :, 0:768]     # q 0:256, k 256:512, v 512:768 (x16 scaled)
    wp8 = w8[:, :, 768:1024]  # proj (x16 scaled)
    smalls = const.tile([128, 8], F32, tag="smalls")
    nc.scalar.dma_start(smalls[:], d["smalls"][:])
    qb16 = smalls[:, 0:2]
    projb = smalls[:, 2:4]
    nw = smalls[:, 4:6]
    nb = smalls[:, 6:8]
    gm = const.tile([128, 128], F32, tag="gm")
    nc.scalar.dma_start(gm[:], d["gm"][:])

    i16 = const.tile([128, 128], BF16, tag="i16")
    nc.scalar.dma_start(i16[:], d["i16"][:])
    ones16 = const.tile([128, 2, 16], FP8, tag="ones16")
    nc.vector.memset(ones16[:], 16.0)
    negoff = const.tile([128, 1], F32, tag="negoff")
    nc.vector.memset(negoff[:], OFFSET)
    onecol = const.tile([1, 128], F32, tag="onecol")
    nc.vector.memset(onecol[:], 1.0)

    # --- phase A: groupnorm stats -> per-chunk scale/bias ---
    stats = const.tile([128, 4], F32, tag="stats")
    scl = const.tile([128, 2], F32, tag="scl")
    bia = const.tile([128, 2], F32, tag="bia")
    gstats_mm = None
    pstats = const.tile([128, 8], F32, tag="pstats")
    with tc.tile_pool(name="scratch", bufs=2) as scr, \
         tc.tile_pool(name="pa_ps", bufs=1, space="PSUM") as pa_ps:
        for t in range(2):
            # stats per DMA-half so each reduce starts as its half lands
            for g in range(2):
                seg = xbf[t][:, g * (N // 2):(g + 1) * (N // 2)]
                nc.vector.reduce_sum(pstats[:, 4 * t + g:4 * t + g + 1], seg,
                                     axis=AX.X)
                sq = scr.tile([128, N // 2], F32, tag="sq")
                nc.scalar.activation(sq[:], seg, AF.Square,
                                     accum_out=pstats[:, 4 * t + 2 + g:4 * t + 3 + g])
            for kind in range(2):
                nc.vector.reduce_sum(stats[:, 2 * t + kind:2 * t + kind + 1],
                                     pstats[:, 4 * t + 2 * kind:4 * t + 2 * kind + 2],
                                     axis=AX.X)
            gstats = pa_ps.tile([128, 2], F32, tag=f"gstats{t}", name=f"gstats{t}")
            gstats_mm = nc.tensor.matmul(gstats[:], gm[:], stats[:, 2 * t:2 * t + 2],
                                         start=True, stop=True)
            mex = const.tile([128, 2], F32, tag=f"mex{t}", name=f"mex{t}")
            nc.vector.tensor_scalar_mul(mex[:], gstats[:], 1.0 / GSIZE)
            mean = mex[:, 0:1]
            ex2 = mex[:, 1:2]
            var = const.tile([128, 1], F32, tag=f"var{t}", name=f"var{t}")
            lnv = const.tile([128, 1], F32, tag=f"lnv{t}", name=f"lnv{t}")
            rstd = const.tile([128, 1], F32, tag=f"rstd{t}", name=f"rstd{t}")
            negm2 = const.tile([128, 1], F32, tag=f"negm2{t}", name=f"negm2{t}")
            nc.vector.scalar_tensor_tensor(negm2[:], mean, -1.0, mean,
                                           op0=ALU.mult, op1=ALU.mult)
            nc.vector.scalar_tensor_tensor(var[:], ex2, EPS, negm2[:],
                                           op0=ALU.add, op1=ALU.add)
            # Sqrt+reciprocal (an exp(-0.5*ln(var)) variant avoids no table
            # loads: the set-selection pass picks minimal per-function sets,
            # so Ln/Exp ping-pong costs MORE loads than Sqrt does)
            nc.scalar.activation(lnv[:], var[:], AF.Sqrt)
            nc.vector.reciprocal(rstd[:], lnv[:])
            nc.vector.tensor_mul(scl[:, t:t + 1], nw[:, t:t + 1], rstd[:])
            mscl = const.tile([128, 1], F32, tag=f"mscl{t}", name=f"mscl{t}")
            nc.vector.tensor_mul(mscl[:], mean, scl[:, t:t + 1])
            nc.vector.tensor_sub(bia[:, t:t + 1], nb[:, t:t + 1], mscl[:])
        # preload the exp table set while phase B runs (one-time ~2.7us)
        warm = const.tile([128, 1], F32, tag="expwarm")
        nc.scalar.activation(warm[:], stats[:, 0:1], AF.Exp, scale=0.0)

    # --- phase B: h (fp8 DR), q/k fp8 DR per-stripe tiles, vT fp8 pairs ---
    # Per-stripe q/k tiles keep the dependency granularity fine enough that
    # phase C's first score matmuls start as soon as phase B's first stripe
    # lands, instead of waiting for all of q/k.
    qt = [const.tile([128, 2, SW], FP8, tag=f"qt{s}", name=f"qt{s}")
          for s in range(NSTRIPE)]
    kt = [const.tile([128, 2, SW], FP8, tag=f"kt{s}", name=f"kt{s}")
          for s in range(NSTRIPE)]
    vtp = []
    for jp in range(NPAIR):
        vtp.append(const.tile([128, 2, 256], FP8, tag=f"vtp{jp}", name=f"vtp{jp}"))

    with tc.tile_pool(name="hpool", bufs=3) as hp, \
         tc.tile_pool(name="pbk_ps", bufs=2, space="PSUM") as pbk, \
         tc.tile_pool(name="pbq_ps", bufs=2, space="PSUM") as pbq, \
         tc.tile_pool(name="pbv_ps", bufs=2, space="PSUM") as pbv:
        for s in range(NSTRIPE):
            sl = slice(s * SW, (s + 1) * SW)
            ht = hp.tile([128, 2, SW], FP8, tag="h", name="h")
            for t in range(2):
                nc.scalar.activation(ht[:, t, :], xbf[t][:, sl], AF.Identity,
                                     bias=bia[:, t:t + 1], scale=scl[:, t:t + 1])
            # k (no bias: it cancels in the softmax over j)
            kps = pbk.tile([128, 1024], F32, tag="kps", name="kps")
            for t in range(2):
                nc.tensor.matmul(kps[:, t * SW:(t + 1) * SW],
                                 wq8[:, :, 256 + t * 128:256 + (t + 1) * 128],
                                 ht[:], start=True, stop=True, perf_mode=DR)
            nc.scalar.activation(kt[s][:, :, :], kps[:], AF.Copy)
            # q (+16*bias)
            for t in range(2):
                qps = pbq.tile([128, SW], F32, tag="qps", name="qps")
                nc.tensor.matmul(qps[:], wq8[:, :, t * 128:(t + 1) * 128],
                                 ht[:], start=True, stop=True, perf_mode=DR)
                nc.vector.tensor_scalar_add(qt[s][:, t, :], qps[:], qb16[:, t:t + 1])
            # vT pairs: two n4-chunks of 128 -> one [128, 512] psum -> one copy
            for half in range(2):
                vps = pbv.tile([128, 512], F32, tag="vps", name="vps")
                for par in range(2):
                    n4 = half * 2 + par
                    nc.tensor.matmul(vps[:, par * 256:(par + 1) * 256],
                                     ht[:, :, n4 * 128:(n4 + 1) * 128],
                                     wq8[:, :, 512:768], start=True, stop=True,
                                     perf_mode=DR)
                jp = s * 2 + half
                nc.vector.tensor_copy(vtp[jp][:, :, :], vps[:])

    # --- phase C: attention + proj + residual, per i-stripe ---
    # PSUM: s_ps 5 banks + a_ps 2 + do_ps 1 (denominator and o_ps share one
    # bank; their lifetimes are disjoint within the stripe-tail rotation).
    with tc.tile_pool(name="wpool", bufs=PLAG + 3) as wpo, \
         tc.tile_pool(name="misc", bufs=3) as mp, \
         tc.tile_pool(name="s_ps", bufs=5, space="PSUM") as spo, \
         tc.tile_pool(name="a_ps", bufs=2, space="PSUM") as apo, \
         tc.tile_pool(name="do_ps", bufs=1, space="PSUM") as dpo:
        opo = dpo

        def make_tail(ist, denom, a_ps):
            sl = slice(ist * SW, (ist + 1) * SW)
            st = {}

            def part1():
                rinv = mp.tile([1, SW], F32, tag="rinv")
                nc.vector.reciprocal_approx_fast(out=rinv[:], in_=denom[:])
                # broadcast rinv to all partitions with a rank-1 f32 matmul
                # into the shared do-bank (its lifetime fits between the
                # denominator read and the o_ps writes)
                rb = mp.tile([128, SW], F32, tag="rb")
                nc.gpsimd.partition_broadcast(rb[:], rinv[:], channels=128)
                st["rb"] = rb

            def part2():
                a8 = mp.tile([128, 2, SW], FP8, tag="a8")
                for ct in range(2):
                    nc.vector.tensor_mul(a8[:, ct, :], a_ps[ct][:], st["rb"][:])
                st["a8"] = a8

            def part3():
                a8 = st["a8"]
                for dt in range(2):
                    o_ps = opo.tile([128, SW], F32, tag="do", name="ops")
                    nc.tensor.matmul(o_ps[:], wp8[:, :, dt * 128:(dt + 1) * 128],
                                     a8[:], start=True, stop=False, perf_mode=DR)
                    # residual: o_ps += 16*xbf via identity matmul (x16 so the
                    # 1/16 proj descale below leaves x unscaled)
                    nc.tensor.matmul(o_ps[:], i16[:], xbf[dt][:, sl],
                                     start=False, stop=True)
                    o_t = mp.tile([128, SW], F32, tag=f"ot{dt}", name=f"ot{dt}")
                    # alternate the bias-descale between ACT and DVE: ACT is
                    # the busier engine, so one of the two per stripe goes DVE
                    if (ist + dt) % 2 == 0:
                        nc.scalar.activation(o_t[:], o_ps[:], AF.Identity,
                                             bias=projb[:, dt:dt + 1], scale=1.0 / 16.0)
                    else:
                        nc.vector.tensor_scalar(o_t[:], o_ps[:], 1.0 / 16.0,
                                                projb[:, dt:dt + 1],
                                                op0=ALU.mult, op1=ALU.add)
                    nc.sync.dma_start(d["out"][dt * 128:(dt + 1) * 128, sl], o_t[:])

            return [part1, part2, part3]

        pending = None
        for ist in range(NSTRIPE):
            isl = slice(ist * SW, (ist + 1) * SW)
            denom = None
            a_ps = None
            wpairs = {}
            for p in range(NPAIR + PLAG):
                if p < NPAIR:
                    wt = wpo.tile([128, 2, SW], FP8, tag="wp", name="wp")
                    for r in range(2):
                        jt = 2 * p + r
                        s_t = spo.tile([128, SW], F32, tag="sps", name="sps")
                        nc.tensor.matmul(s_t[:],
                                         kt[jt // 4][:, :, (jt % 4) * 128:(jt % 4 + 1) * 128],
                                         qt[ist][:], start=True, stop=True,
                                         perf_mode=DR)
                        if jt in ACT_JTS:
                            nc.scalar.activation(wt[:, r, :], s_t[:], AF.Exp,
                                                 bias=negoff[:], scale=SC2)
                        else:
                            nc.vector.tensor_scalar(
                                wt[:, r, :].bitcast(U8), s_t[:],
                                SCHRA_A * SC2, 56.0 + OFFSET * SCHRA_A,
                                op0=ALU.mult, op1=ALU.add)
                    wpairs[p] = wt
                if pending is not None:
                    if p == 0:
                        pending[0]()
                    elif p == 1:
                        pending[1]()
                    elif p == 2:
                        pending[2]()
                        pending = None
                if p == 3:
                    # allocated here (not at stripe start) so the shared
                    # den/o_ps bank's rotation order matches temporal order
                    denom = dpo.tile([1, SW], F32, tag="do", name="den")
                if p >= PLAG:
                    p2 = p - PLAG
                    w2 = wpairs.pop(p2)
                    if a_ps is None:
                        a_ps = [apo.tile([128, SW], F32, tag="aps", name="aps")
                                for _ in range(2)]
                    nc.tensor.matmul(denom[:], ones16[:, :, 0:1], w2[:],
                                     start=(p2 == 0), stop=(p2 == NPAIR - 1),
                                     perf_mode=DR)
                    for ct in range(2):
                        nc.tensor.matmul(a_ps[ct][:],
                                         vtp[p2][:, :, ct * 128:(ct + 1) * 128],
                                         w2[:], start=(p2 == 0),
                                         stop=(p2 == NPAIR - 1), perf_mode=DR)
            pending = make_tail(ist, denom, a_ps)
        for part in pending:
            part()

    const.release()


def build_program(repeat: int = 1):
    nc = bacc.Bacc("TRN2", target_bir_lowering=False, debug=False, num_devices=8)
    d = {
        "xbf": nc.declare_dram_parameter("xbf", [C, N], BF16, isOutput=False),
        "w8": nc.declare_dram_parameter("w8", [128, 2048], U8, isOutput=False),
        "smalls": nc.declare_dram_parameter("smalls", [128, 8], F32, isOutput=False),
        "gm": nc.declare_dram_parameter("gm", [128, 128], F32, isOutput=False),
        "i16": nc.declare_dram_parameter("i16", [128, 128], BF16, isOutput=False),
        "out": nc.declare_dram_parameter("out", [C, N], F32, isOutput=True),
    }
    with tile.TileContext(nc) as tc:
        for _ in range(repeat):
            _emit(nc, tc, d)
    nc.compile()
    return nc


def make_in_maps(x, norm_w, norm_b, qkv_w, qkv_b, proj_w, proj_b):
    x = np.asarray(x, np.float32)
    B = x.shape[0]
    qkv_w = np.asarray(qkv_w, np.float32)
    qkv_b = np.asarray(qkv_b, np.float32)
    proj_w = np.asarray(proj_w, np.float32)
    proj_b = np.asarray(proj_b, np.float32)
    FP8NP = ml_dtypes.float8_e4m3

    qkvT = (16.0 * qkv_w).T                      # [256, 768]
    projT = (16.0 * proj_w).T                    # [256, 256]
    wall = np.concatenate([qkvT, projT], axis=1)  # [256, 1024]
    w8 = wall.reshape(2, 128, 1024).transpose(1, 0, 2)  # [p, r, d]
    w8 = np.ascontiguousarray(w8.astype(FP8NP).view(np.uint8).reshape(128, 2048))

    projb_f = proj_b + proj_w @ qkv_b[2 * C:]    # fold v-bias into proj bias
    smalls = np.zeros((128, 8), np.float32)
    smalls[:, 0:2] = (16.0 * qkv_b[:C]).reshape(2, 128).T
    smalls[:, 2:4] = projb_f.reshape(2, 128).T
    smalls[:, 4:6] = np.asarray(norm_w, np.float32).reshape(2, 128).T
    smalls[:, 6:8] = np.asarray(norm_b, np.float32).reshape(2, 128).T
    shared = {
        "w8": w8,
        "smalls": smalls,
        "gm": (np.arange(128)[:, None] // 8 == np.arange(128)[None, :] // 8).astype(np.float32),
        "i16": (16.0 * np.eye(128, dtype=np.float32)).astype(ml_dtypes.bfloat16),
    }
    return [
        dict(shared,
             xbf=np.ascontiguousarray(x[b].reshape(C, N)).astype(ml_dtypes.bfloat16))
        for b in range(B)
    ]


_NC_CACHE = {}


def get_program(repeat: int = 1):
    if repeat not in _NC_CACHE:
        _NC_CACHE[repeat] = build_program(repeat)
    return _NC_CACHE[repeat]


def kernel(x, norm_w, norm_b, qkv_w, qkv_b, proj_w, proj_b):
    x = np.asarray(x, np.float32)
    B, C_, H_, W_ = x.shape
    in_maps = make_in_maps(x, norm_w, norm_b, qkv_w, qkv_b, proj_w, proj_b)
    nc = get_program()
    res = run_bass_kernel_spmd(nc, in_maps, core_ids=list(range(len(in_maps))))
    out = np.stack([np.asarray(res.results[b]["out"], np.float32) for b in range(B)])
    return out.reshape(B, C_, H_, W_)


# revision 49
# speedup vs baseline: 1.1425x; 1.1425x over previous
"""Trainium2 Bass kernel for nn_AttentionBlock (GroupNorm + spatial self-attention
+ residual). Full inputs in, full outputs out; data-parallel over batch (B=8)
across 8 NeuronCores; each core processes one [C=256, N=4096] image.

Design (full-fp8 attention; see study.py for the precision budget):
  - All big matmuls run fp8-e4m3 DoubleRow (0.5 cycles/row, 4x less PE time
    than two bf16 chunk matmuls for a 256-contraction). Weights are scaled
    x16 on the host so their sigma~1 lands in fp8's normal range; the score
    scale absorbs 1/256 and the softmax denominator absorbs the v-side 16.
  - DR layouts are [128, 2, n] with channel = r*128 + p.
  - q = 16(q_mm + bq) fp8; k = 16 k_mm fp8 (k-bias provably cancels in the
    softmax over j); v = 16 v_mm fp8 transposed to [n, c] pairs (v-bias passes
    through softmax and is folded into proj_b on the host).
  - scores s' = k'^T q' = 256 s, PSUM f32; w = exp(s'*SCALE/256 - 4) in fp8:
    the e^-4 offset keeps fp8 in range and cancels in normalization. exp runs
    split across two engines: ACT (exact exp, fp8 out) for the first
    N_ACT_PAIRS j-pairs of each stripe, DVE for the rest via a Schraudolph
    bit-trick (uint8 = rne(8/ln2*u + 56) is the fp8-e4m3 encoding of e^u;
    DVE's f32->uint8 convert rounds+saturates, verified on HW).
  - softmax denominator: DR ones-matmul (value 16 = v descale) accumulating
    [1, SW] in PSUM; reciprocal_approx_fast on DVE; partition_broadcast on
    the (otherwise idle) Pool engine; a = a_ps * rinv -> fp8 pairs (DVE).
  - proj fp8 DR with the residual accumulated into the same PSUM group via a
    16*I bf16 identity-matmul of xbf; o = Identity(o_ps/16 + projb') on ACT
    writes the final output, DMA'd on the SP HWDGE queue.
  - PSUM: 5 per-jt score banks + 2 AV banks + 1 bank shared by the
    denominator and o_ps (temporally disjoint, same pool tag). PE writes into
    a shared bank via a second aliased matmul fail BIR verification, so the
    rinv broadcast stays on Pool rather than a PE rank-1 matmul.
"""

import sys

try:
    import concourse  # noqa: F401
except ImportError:
    sys.path.insert(0, "/opt/trn_rl_repo")

import numpy as np
import ml_dtypes

import bass_rust as _bass_rust
import concourse.bacc as bacc
import concourse.tile as tile
from concourse import mybir
from concourse import bass_isa
from concourse.bass_utils import run_bass_kernel_spmd

F32 = mybir.dt.float32
BF16 = mybir.dt.bfloat16
FP8 = mybir.dt.float8e4
U8 = mybir.dt.uint8
AF = mybir.ActivationFunctionType
ALU = mybir.AluOpType
AX = mybir.AxisListType
DR = mybir.MatmulPerfMode.DoubleRow

C = 256          # channels
N = 4096         # spatial positions
GROUPS = 32
EPS = 1e-5
SCALE = C ** -0.5
SC2 = SCALE / 256.0          # fold the x16 q/k weight scaling out of scores
OFFSET = -4.0                # exp offset; cancels in softmax normalization
SCHRA_A = 8.0 / np.log(2.0)  # fp8-e4m3 schraudolph slope
NSTRIPE = 8
SW = N // NSTRIPE            # 512
NPAIR = N // 256             # 16 j-pairs (j-tiles of 128, two per pair)
PLAG = 8                      # pairs of lag between exp production and AV/ones
NJT = N // 128               # 32 j-tiles per stripe
# exp engine per j-tile, spread so ACT (faster, ~18 tiles) and DVE (~14) run
# concurrently within a stripe (a contiguous split serializes them through
# the score-buffer WAR).
ACT_JTS = frozenset(j for j in range(NJT) if (j * 18) // NJT != ((j + 1) * 18) // NJT)
GSIZE = (C // GROUPS) * N


def _emit(nc, tc, d):
    const = tc.alloc_tile_pool(name="const", bufs=1)

    # --- input DMAs: bf16 x split across both HWDGE queues, then weights ---
    xbf = []
    for t in range(2):
        xb_ = const.tile([128, N], BF16, tag=f"xbf{t}", name=f"xbf{t}")
        for g, eng in enumerate((nc.sync, nc.scalar)):
            eng.dma_start(xb_[:, g * (N // 2):(g + 1) * (N // 2)],
                          d["xbf"][t * 128:(t + 1) * 128,
                                   g * (N // 2):(g + 1) * (N // 2)])
        xbf.append(xb_)

    w8 = const.tile([128, 2, 1024], FP8, tag="w8")
    nc.scalar.dma_start(w8[:].bitcast(U8), d["w8"][:])
    wq8 = w8[:, :, 0:768]     # q 0:256, k 256:512, v 512:768 (x16 scaled)
    wp8 = w8[:, :, 768:1024]  # proj (x16 scaled)
    smalls = const.tile([128, 8], F32, tag="smalls")
    nc.scalar.dma_start(smalls[:], d["smalls"][:])
    qb16 = smalls[:, 0:2]
    projb = smalls[:, 2:4]
    nw = smalls[:, 4:6]
    nb = smalls[:, 6:8]
    gm = const.tile([128, 128], F32, tag="gm")
    nc.scalar.dma_start(gm[:], d["gm"][:])

    i16 = const.tile([128, 128], BF16, tag="i16")
    nc.scalar.dma_start(i16[:], d["i16"][:])
    ones16 = const.tile([128, 2, 16], FP8, tag="ones16")
    nc.vector.memset(ones16[:], 16.0)
    negoff = const.tile([128, 1], F32, tag="negoff")
    nc.vector.memset(negoff[:], OFFSET)
    onecol = const.tile([1, 128], F32, tag="onecol")
    nc.vector.memset(onecol[:], 1.0)

    # --- phase A: groupnorm stats -> per-chunk scale/bias ---
    stats = const.tile([128, 4], F32, tag="stats")
    scl = const.tile([128, 2], F32, tag="scl")
    bia = const.tile([128, 2], F32, tag="bia")
    gstats_mm = None
    pstats = const.tile([128, 8], F32, tag="pstats")
    with tc.tile_pool(name="scratch", bufs=2) as scr, \
         tc.tile_pool(name="pa_ps", bufs=1, space="PSUM") as pa_ps:
        for t in range(2):
            # stats per DMA-half so each reduce starts as its half lands
            for g in range(2):
                seg = xbf[t][:, g * (N // 2):(g + 1) * (N // 2)]
                nc.vector.reduce_sum(pstats[:, 4 * t + g:4 * t + g + 1], seg,
                                     axis=AX.X)
                sq = scr.tile([128, N // 2], F32, tag="sq")
                nc.scalar.activation(sq[:], seg, AF.Square,
                                     accum_out=pstats[:, 4 * t + 2 + g:4 * t + 3 + g])
            for kind in range(2):
                nc.vector.reduce_sum(stats[:, 2 * t + kind:2 * t + kind + 1],
                                     pstats[:, 4 * t + 2 * kind:4 * t + 2 * kind + 2],
                                     axis=AX.X)
            gstats = pa_ps.tile([128, 2], F32, tag=f"gstats{t}", name=f"gstats{t}")
            gstats_mm = nc.tensor.matmul(gstats[:], gm[:], stats[:, 2 * t:2 * t + 2],
                                         start=True, stop=True)
            mex = const.tile([128, 2], F32, tag=f"mex{t}", name=f"mex{t}")
            nc.vector.tensor_scalar_mul(mex[:], gstats[:], 1.0 / GSIZE)
            mean = mex[:, 0:1]
            ex2 = mex[:, 1:2]
            var = const.tile([128, 1], F32, tag=f"var{t}", name=f"var{t}")
            lnv = const.tile([128, 1], F32, tag=f"lnv{t}", name=f"lnv{t}")
            rstd = const.tile([128, 1], F32, tag=f"rstd{t}", name=f"rstd{t}")
            negm2 = const.tile([128, 1], F32, tag=f"negm2{t}", name=f"negm2{t}")
            nc.vector.scalar_tensor_tensor(negm2[:], mean, -1.0, mean,
                                           op0=ALU.mult, op1=ALU.mult)
            nc.vector.scalar_tensor_tensor(var[:], ex2, EPS, negm2[:],
                                           op0=ALU.add, op1=ALU.add)
            # Sqrt+reciprocal (an exp(-0.5*ln(var)) variant avoids no table
            # loads: the set-selection pass picks minimal per-function sets,
            # so Ln/Exp ping-pong costs MORE loads than Sqrt does)
            nc.scalar.activation(lnv[:], var[:], AF.Sqrt)
            nc.vector.reciprocal(rstd[:], lnv[:])
            nc.vector.tensor_mul(scl[:, t:t + 1], nw[:, t:t + 1], rstd[:])
            mscl = const.tile([128, 1], F32, tag=f"mscl{t}", name=f"mscl{t}")
            nc.vector.tensor_mul(mscl[:], mean, scl[:, t:t + 1])
            nc.vector.tensor_sub(bia[:, t:t + 1], nb[:, t:t + 1], mscl[:])
        # preload the exp table set while phase B runs (one-time ~2.7us)
        warm = const.tile([128, 1], F32, tag="expwarm")
        nc.scalar.activation(warm[:], stats[:, 0:1], AF.Exp, scale=0.0)

    # --- phase B: h (fp8 DR), q/k fp8 DR per-stripe tiles, vT fp8 pairs ---
    # Per-stripe q/k tiles keep the dependency granularity fine enough that
    # phase C's first score matmuls start as soon as phase B's first stripe
    # lands, instead of waiting for all of q/k.
    qt = [const.tile([128, 2, SW], FP8, tag=f"qt{s}", name=f"qt{s}")
          for s in range(NSTRIPE)]
    kt = [const.tile([128, 2, SW], FP8, tag=f"kt{s}", name=f"kt{s}")
          for s in range(NSTRIPE)]
    vtp = []
    for jp in range(NPAIR):
        vtp.append(const.tile([128, 2, 256], FP8, tag=f"vtp{jp}", name=f"vtp{jp}"))

    with tc.tile_pool(name="hpool", bufs=3) as hp, \
         tc.tile_pool(name="pbk_ps", bufs=2, space="PSUM") as pbk, \
         tc.tile_pool(name="pbq_ps", bufs=2, space="PSUM") as pbq, \
         tc.tile_pool(name="pbv_ps", bufs=2, space="PSUM") as pbv:
        for s in range(NSTRIPE):
            sl = slice(s * SW, (s + 1) * SW)
            ht = hp.tile([128, 2, SW], FP8, tag="h", name="h")
            for t in range(2):
                nc.scalar.activation(ht[:, t, :], xbf[t][:, sl], AF.Identity,
                                     bias=bia[:, t:t + 1], scale=scl[:, t:t + 1])
            # k (no bias: it cancels in the softmax over j)
            kps = pbk.tile([128, 1024], F32, tag="kps", name="kps")
            for t in range(2):
                nc.tensor.matmul(kps[:, t * SW:(t + 1) * SW],
                                 wq8[:, :, 256 + t * 128:256 + (t + 1) * 128],
                                 ht[:], start=True, stop=True, perf_mode=DR)
            nc.scalar.activation(kt[s][:, :, :], kps[:], AF.Copy)
            # q (+16*bias)
            for t in range(2):
                qps = pbq.tile([128, SW], F32, tag="qps", name="qps")
                nc.tensor.matmul(qps[:], wq8[:, :, t * 128:(t + 1) * 128],
                                 ht[:], start=True, stop=True, perf_mode=DR)
                nc.vector.tensor_scalar_add(qt[s][:, t, :], qps[:], qb16[:, t:t + 1])
            # vT pairs: two n4-chunks of 128 -> one [128, 512] psum -> one copy
            for half in range(2):
                vps = pbv.tile([128, 512], F32, tag="vps", name="vps")
                for par in range(2):
                    n4 = half * 2 + par
                    nc.tensor.matmul(vps[:, par * 256:(par + 1) * 256],
                                     ht[:, :, n4 * 128:(n4 + 1) * 128],
                                     wq8[:, :, 512:768], start=True, stop=True,
                                     perf_mode=DR)
                jp = s * 2 + half
                nc.vector.tensor_copy(vtp[jp][:, :, :], vps[:])

    # --- phase C: attention + proj + residual, per i-stripe ---
    # PSUM: s_ps 5 banks + a_ps 2 + do_ps 1 (denominator and o_ps share one
    # bank; their lifetimes are disjoint within the stripe-tail rotation).
    with tc.tile_pool(name="wpool", bufs=PLAG + 3) as wpo, \
         tc.tile_pool(name="misc", bufs=3) as mp, \
         tc.tile_pool(name="s_ps", bufs=5, space="PSUM") as spo, \
         tc.tile_pool(name="a_ps", bufs=2, space="PSUM") as apo, \
         tc.tile_pool(name="do_ps", bufs=1, space="PSUM") as dpo:
        opo = dpo

        def make_tail(ist, denom, a_ps):
            sl = slice(ist * SW, (ist + 1) * SW)
            st = {}

            def part1():
                rinv = mp.tile([1, SW], F32, tag="rinv")
                nc.vector.reciprocal_approx_fast(out=rinv[:], in_=denom[:])
                # broadcast rinv to all partitions with a rank-1 f32 matmul
                # into the shared do-bank (its lifetime fits between the
                # denominator read and the o_ps writes)
                rb = mp.tile([128, SW], F32, tag="rb")
                nc.gpsimd.partition_broadcast(rb[:], rinv[:], channels=128)
                st["rb"] = rb

            def part2():
                a8 = mp.tile([128, 2, SW], FP8, tag="a8")
                for ct in range(2):
                    nc.vector.tensor_mul(a8[:, ct, :], a_ps[ct][:], st["rb"][:])
                st["a8"] = a8

            def part3():
                a8 = st["a8"]
                for dt in range(2):
                    o_ps = opo.tile([128, SW], F32, tag="do", name="ops")
                    nc.tensor.matmul(o_ps[:], wp8[:, :, dt * 128:(dt + 1) * 128],
                                     a8[:], start=True, stop=False, perf_mode=DR)
                    # residual: o_ps += 16*xbf via identity matmul (x16 so the
                    # 1/16 proj descale below leaves x unscaled)
                    nc.tensor.matmul(o_ps[:], i16[:], xbf[dt][:, sl],
                                     start=False, stop=True)
                    o_t = mp.tile([128, SW], F32, tag=f"ot{dt}", name=f"ot{dt}")
                    # alternate the bias-descale between ACT and DVE: ACT is
                    # the busier engine, so one of the two per stripe goes DVE
                    if (ist + dt) % 2 == 0:
                        nc.scalar.activation(o_t[:], o_ps[:], AF.Identity,
                                             bias=projb[:, dt:dt + 1], scale=1.0 / 16.0)
                    else:
                        nc.vector.tensor_scalar(o_t[:], o_ps[:], 1.0 / 16.0,
                                                projb[:, dt:dt + 1],
                                                op0=ALU.mult, op1=ALU.add)
                    nc.sync.dma_start(d["out"][dt * 128:(dt + 1) * 128, sl], o_t[:])

            return [part1, part2, part3]

        pending = None
        for ist in range(NSTRIPE):
            isl = slice(ist * SW, (ist + 1) * SW)
            denom = None
            a_ps = None
            wpairs = {}
            for p in range(NPAIR + PLAG):
                if p < NPAIR:
                    wt = wpo.tile([128, 2, SW], FP8, tag="wp", name="wp")
                    for r in range(2):
                        jt = 2 * p + r
                        s_t = spo.tile([128, SW], F32, tag="sps", name="sps")
                        nc.tensor.matmul(s_t[:],
                                         kt[jt // 4][:, :, (jt % 4) * 128:(jt % 4 + 1) * 128],
                                         qt[ist][:], start=True, stop=True,
                                         perf_mode=DR)
                        if jt in ACT_JTS:
                            nc.scalar.activation(wt[:, r, :], s_t[:], AF.Exp,
                                                 bias=negoff[:], scale=SC2)
                        else:
                            nc.vector.tensor_scalar(
                                wt[:, r, :].bitcast(U8), s_t[:],
                                SCHRA_A * SC2, 56.0 + OFFSET * SCHRA_A,
                                op0=ALU.mult, op1=ALU.add)
                    wpairs[p] = wt
                if pending is not None:
                    if p == 0:
                        pending[0]()
                    elif p == 1:
                        pending[1]()
                    elif p == 2:
                        pending[2]()
                        pending = None
                if p == 3:
                    # allocated here (not at stripe start) so the shared
                    # den/o_ps bank's rotation order matches temporal order
                    denom = dpo.tile([1, SW], F32, tag="do", name="den")
                if p >= PLAG:
                    p2 = p - PLAG
                    w2 = wpairs.pop(p2)
                    if a_ps is None:
                        a_ps = [apo.tile([128, SW], F32, tag="aps", name="aps")
                                for _ in range(2)]
                    nc.tensor.matmul(denom[:], ones16[:, :, 0:1], w2[:],
                                     start=(p2 == 0), stop=(p2 == NPAIR - 1),
                                     perf_mode=DR)
                    for ct in range(2):
                        nc.tensor.matmul(a_ps[ct][:],
                                         vtp[p2][:, :, ct * 128:(ct + 1) * 128],
                                         w2[:], start=(p2 == 0),
                                         stop=(p2 == NPAIR - 1), perf_mode=DR)
            pending = make_tail(ist, denom, a_ps)
        for part in pending:
            part()

    const.release()


def build_program(repeat: int = 1):
    nc = bacc.Bacc("TRN2", target_bir_lowering=False, debug=False, num_devices=8)
    d = {
        "xbf": nc.declare_dram_parameter("xbf", [C, N], BF16, isOutput=False),
        "w8": nc.declare_dram_parameter("w8", [128, 2048], U8, isOutput=False),
        "smalls": nc.declare_dram_parameter("smalls", [128, 8], F32, isOutput=False),
        "gm": nc.declare_dram_parameter("gm", [128, 128], F32, isOutput=False),
        "i16": nc.declare_dram_parameter("i16", [128, 128], BF16, isOutput=False),
        "out": nc.declare_dram_parameter("out", [C, N], F32, isOutput=True),
    }
    with tile.TileContext(nc) as tc:
        for _ in range(repeat):
            _emit(nc, tc, d)
    nc.compile()
    return nc


def make_in_maps(x, norm_w, norm_b, qkv_w, qkv_b, proj_w, proj_b):
    x = np.asarray(x, np.float32)
    B = x.shape[0]
    qkv_w = np.asarray(qkv_w, np.float32)
    qkv_b = np.asarray(qkv_b, np.float32)
    proj_w = np.asarray(proj_w, np.float32)
    proj_b = np.asarray(proj_b, np.float32)
    FP8NP = ml_dtypes.float8_e4m3

    qkvT = (16.0 * qkv_w).T                      # [256, 768]
    projT = (16.0 * proj_w).T                    # [256, 256]
    wall = np.concatenate([qkvT, projT], axis=1)  # [256, 1024]
    w8 = wall.reshape(2, 128, 1024).transpose(1, 0, 2)  # [p, r, d]
    w8 = np.ascontiguousarray(w8.astype(FP8NP).view(np.uint8).reshape(128, 2048))

    projb_f = proj_b + proj_w @ qkv_b[2 * C:]    # fold v-bias into proj bias
    smalls = np.zeros((128, 8), np.float32)
    smalls[:, 0:2] = (16.0 * qkv_b[:C]).reshape(2, 128).T
    smalls[:, 2:4] = projb_f.reshape(2, 128).T
    smalls[:, 4:6] = np.asarray(norm_w, np.float32).reshape(2, 128).T
    smalls[:, 6:8] = np.asarray(norm_b, np.float32).reshape(2, 128).T
    shared = {
        "w8": w8,
        "smalls": smalls,
        "gm": (np.arange(128)[:, None] // 8 == np.arange(128)[None, :] // 8).astype(np.float32),
        "i16": (16.0 * np.eye(128, dtype=np.float32)).astype(ml_dtypes.bfloat16),
    }
    return [
        dict(shared,
             xbf=np.ascontiguousarray(x[b].reshape(C, N)).astype(ml_dtypes.bfloat16))
        for b in range(B)
    ]


_NC_CACHE = {}


def get_program(repeat: int = 1):
    if repeat not in _NC_CACHE:
        _NC_CACHE[repeat] = build_program(repeat)
    return _NC_CACHE[repeat]


def kernel(x, norm_w, norm_b, qkv_w, qkv_b, proj_w, proj_b):
    x = np.asarray(x, np.float32)
    B, C_, H_, W_ = x.shape
    in_maps = make_in_maps(x, norm_w, norm_b, qkv_w, qkv_b, proj_w, proj_b)
    nc = get_program()
    res = run_bass_kernel_spmd(nc, in_maps, core_ids=list(range(len(in_maps))))
    out = np.stack([np.asarray(res.results[b]["out"], np.float32) for b in range(B)])
    return out.reshape(B, C_, H_, W_)
